# revision 29
# baseline (speedup 1.0000x reference)
"""ConvSTFT on Trainium2: strided conv of x[32, 480000] against a fixed
[514, 1, 400] Fourier basis, hop 100 -> out [32, 514, 4803] f32.

Sharding: pure data parallel. Batch dim (32) split 4-per-core across 8
NeuronCores; the small weight is replicated.

Host prep (sharding layer): pad x by 300 on both sides, then lay it out
hop-transposed:
    x_dev[b, r, f'] = x_padded[b, 100 f' + r]   (r in 0..99)
so each batch loads as a handful of [100-partition x ~2KB-contiguous]
DMAs (~100 descriptors per piece). The weight is passed tap-transposed
and K-padded to 128 rows per hop-phase: wt_p[128 j + r, c] =
weight[c, 0, 100 j + r] (r < 100; rest zero). Both cast to bf16.

Per-core device kernel (Bass/Tile) -- frame-major matmul decomposition:
  t = 100j + r (j in 0..3) turns the overlapped conv into 4
  PSUM-accumulated matmuls per output tile, with FRAMES in the
  stationary/partition (M) dim and ALL 514 channels in the moving (N)
  dim:
      out[f, c] = sum_j sum_r XT[r, f + j] * wt_p[128 j + r, c]
  - lhsT = XT[:, f-tile] (K=128 with zeroed pad rows, M<=128 frames;
    consecutive j are column-shifted views of the same flat SBUF
    buffer), rhs = wsb[:, j, c-half] (N=257; c split 257+257 across two
    PSUM banks since 514 f32 > one 2KB bank).
  - 4j x 514c = 2056 PE columns per 128-frame tile is the floor for
    this conv (400 taps with hop-100 shifts need 4 accumulation steps;
    channels sit in N with zero pad), vs 5 c-tiles x 4j x 512 = 10240
    per 512-frame group for the channel-major layout whose M=2
    leftover c-tile wastes a full stream.
  - K padded to 128 (host-zeroed rows) keeps the LDWEIGHTS pull-ahead
    under the 107ns N=257 stream so back-to-back matmuls stay at the
    column floor (measured 110ns/MM steady-state, the NX issue floor).
  - PSUM evacuated by DVE (half A) / ACT (half B) as f32->bf16 casts
    into [128, 2, 514] bf16 SBUF tiles; every second f-tile issues one
    fully contiguous 263KB store (bf16 halves HBM store traffic to
    ~150 GB/s -- f32 stores saturate the 358 GB/s core bandwidth and
    stall the PE; output quantization adds ~0.1% error vs the 2e-2
    budget).  Pair-stores keep the sync engine at ~350ns/2-ftiles so
    stores never lag and the tail drain stays short.
  - Engine split keeps the PE stream gapless (HAM stays at K=8/8):
    vector = copy A only, scalar = copy B + next-batch loads (each a
    ~128-descriptor DGE), sync = pair stores.  ~26 warmup matmuls keep
    the PE busy from the preamble until the first weight pieces land so
    the HAM clock gate lifts as the real stream starts.
    Measured ~152-154us/core (vs 182us for the channel-major baseline);
    the PE stream runs start-to-finish with no >250ns gaps.
  The device writes out[b, gg, f, h, c] bf16 ([4, 19, 128, 2, 514]);
  the host reorders to frames = 256 gg + 128 h + f, drops the 61 pad
  frames, transposes to [4, 514, 4803] and upcasts to f32 (host-side,
  mirroring the host-side input prep).
"""

import numpy as np
import ml_dtypes

WIN, HOP, C = 400, 100, 514
B, T = 32, 480000
PAD = WIN - HOP                       # 300
N_CORES = 8
B_LOC = B // N_CORES                  # 4
T_PAD = T + 2 * PAD                   # 480600
N_FRAMES = (T_PAD - WIN) // HOP + 1   # 4803
S_BLOCKS = -(-(T_PAD // HOP) // 128)  # 38
N_CHUNKS = S_BLOCKS * 128             # 4864
NJ = WIN // HOP                       # 4
KP = 128                              # zero-padded contraction dim

F_TILE = 128                          # frames per tile (M, partition dim)
N_FT = S_BLOCKS                       # 38 f-tiles
N_FT2 = N_FT // 2                     # 19 store pairs
C_HALF = 257                          # N per matmul (two PSUM banks)


def build_program(b_loc=B_LOC, n_chunks=N_CHUNKS, n_frames=N_FRAMES):
    import concourse.bacc as bacc
    import concourse.mybir as mybir
    import concourse.tile as tile

    dt = mybir.dt
    n_ft = -(-(n_frames) // F_TILE)
    assert n_ft % 2 == 0
    assert n_frames + NJ - 1 <= n_chunks

    nc = bacc.Bacc("TRN2", target_bir_lowering=False, debug=False)
    x_d = nc.dram_tensor(
        "x", [b_loc, KP, n_chunks], dt.bfloat16, kind="ExternalInput"
    ).ap()
    w_d = nc.dram_tensor("wt", [NJ * KP, C], dt.bfloat16, kind="ExternalInput").ap()
    o_d = nc.dram_tensor(
        "out", [b_loc, n_ft // 2, F_TILE, 2, C], dt.bfloat16, kind="ExternalOutput"
    ).ap()

    half = n_chunks // 2

    with tile.TileContext(nc) as tc:
        with (
            tc.tile_pool(name="const", bufs=1) as constp,
            tc.tile_pool(name="obuf", bufs=14) as obufp,
            tc.tile_pool(name="mmps", bufs=8, space="PSUM") as mmps,
        ):
            # Short warmup matmuls open the HAM clock gate while the
            # critical first loads are in flight.
            warm = constp.tile([128, 128], dt.bfloat16)
            nc.vector.memset(warm[:], 0.0)
            # ~26 x 107ns keeps the PE continuously busy from ~7.5us until
            # the first weight pieces land (~10.4us), so the HAM clock
            # gate lifts right as the real stream starts instead of 5us in
            wps = mmps.tile([128, 512], dt.float32, tag="ps")
            for _ in range(26):
                nc.tensor.matmul(wps[0:16, 0:128], warm[:, 0:16], warm[:, :])

            # static per-batch XT buffers; K-pad rows are zeroed host-side
            wsb = constp.tile([KP, NJ, C], dt.bfloat16)
            xts = [
                constp.tile([KP, n_chunks], dt.bfloat16, name=f"xt{i}")
                for i in range(b_loc)
            ]

            # critical first loads split across both HWDGE rings in
            # consumption order: the c-half A weight pieces (66KB each,
            # feeding the psA groups) plus the first xt piece on sync
            # (whose queue starts ~1us earlier), the c-half B pieces on
            # scalar.  Each load is ~128 descriptors of >=0.5KB.
            nc.sync.dma_start(wsb[:, 0, 0:C_HALF], w_d[0:KP, 0:C_HALF])
            nc.scalar.dma_start(wsb[:, 0, C_HALF:C], w_d[0:KP, C_HALF:C])
            nc.sync.dma_start(xts[0][:, 0:640], x_d[0, :, 0:640])
            for j in range(1, NJ):
                wj = w_d[KP * j : KP * (j + 1)]
                nc.sync.dma_start(wsb[:, j, 0:C_HALF], wj[:, 0:C_HALF])
                nc.scalar.dma_start(wsb[:, j, C_HALF:C], wj[:, C_HALF:C])
            pieces = [
                (c0, min(c0 + 1088, n_chunks))
                for c0 in range(640, n_chunks, 1088)
            ]
            for i, (c0, c1) in enumerate(pieces):
                eng = nc.scalar if i % 2 == 0 else nc.sync
                eng.dma_start(xts[0][:, c0:c1], x_d[0, :, c0:c1])

            for b in range(b_loc):
                xt = xts[b]
                for g in range(n_ft):
                    f0 = g * F_TILE
                    fm = min(F_TILE, n_frames - f0)  # last tile: 67

                    # next batch's input, two pieces, issued mid-batch on
                    # scalar (cheap DGE; stream trickles behind copies)
                    if b + 1 < b_loc and g in (10, 24):
                        lo = 0 if g == 10 else half
                        hi = half if g == 10 else n_chunks
                        nc.scalar.dma_start(
                            xts[b + 1][:, lo:hi], x_d[b + 1, :, lo:hi]
                        )

                    if g % 2 == 0:
                        obuf = obufp.tile([F_TILE, 2, C], dt.bfloat16, tag="ob")
                    oh = obuf[0:fm, g % 2, :]

                    psA = mmps.tile([128, 512], dt.float32, tag="ps")
                    for j in range(NJ):
                        nc.tensor.matmul(
                            psA[0:fm, 0:C_HALF],
                            xt[:, f0 + j : f0 + j + fm],
                            wsb[:, j, 0:C_HALF],
                            start=(j == 0),
                            stop=(j == NJ - 1),
                        )
                    nc.vector.tensor_copy(oh[:, 0:C_HALF], psA[0:fm, 0:C_HALF])
                    if b == b_loc - 1 and g == n_ft - 1:
                        # final f-tile: ship the A half as soon as its copy
                        # lands so only a 67KB store trails the last matmul
                        nc.sync.dma_start(
                            o_d[b, g // 2, :, 1:2, 0:C_HALF],
                            obuf[:, 1:2, 0:C_HALF],
                        )

                    psB = mmps.tile([128, 512], dt.float32, tag="ps")
                    for j in range(NJ):
                        nc.tensor.matmul(
                            psB[0:fm, 0:C_HALF],
                            xt[:, f0 + j : f0 + j + fm],
                            wsb[:, j, C_HALF:C],
                            start=(j == 0),
                            stop=(j == NJ - 1),
                        )
                    nc.scalar.copy(oh[:, C_HALF:C], psB[0:fm, 0:C_HALF])

                    if b == b_loc - 1 and g == n_ft - 1:
                        nc.sync.dma_start(
                            o_d[b, g // 2, :, 1:2, C_HALF:C],
                            obuf[:, 1:2, C_HALF:C],
                        )
                    elif b == b_loc - 1 and g == n_ft - 2:
                        # split the final pair so the tail drain is short
                        nc.sync.dma_start(
                            o_d[b, g // 2, :, 0:1, :], obuf[:, 0:1, :]
                        )
                    elif g % 2 == 1:
                        nc.sync.dma_start(o_d[b, g // 2], obuf[:])

    nc.compile()
    return nc


_NC = None
LAST_RESULTS = None


def _ensure_axon_hooks_stub():
    """If BASS_TRACE is set but the container's antenv lacks axon_hooks,
    run_bass_kernel_spmd would crash on import; degrade to no-trace."""
    import sys

    try:
        import antenv.axon_hooks  # noqa: F401
    except ImportError:
        import types

        import antenv

        m = types.ModuleType("antenv.axon_hooks")
        m.get_axon_ntff_profile_hook = lambda: None
        m.set_axon_ntff_profile_hook = lambda h: None
        sys.modules["antenv.axon_hooks"] = m
        antenv.axon_hooks = m


def _prep_inputs(x, weight):
    x = np.asarray(x, dtype=np.float32)
    w = np.asarray(weight, dtype=np.float32)
    nb = x.shape[0]
    xp = np.zeros((nb, N_CHUNKS * HOP), dtype=np.float32)
    xp[:, PAD : PAD + x.shape[1]] = x
    # hop-transpose: [b, f', r] -> [b, r, f'], K-pad rows 100..127 with 0
    xdev = np.zeros((nb, KP, N_CHUNKS), dtype=ml_dtypes.bfloat16)
    xdev[:, :HOP, :] = xp.reshape(nb, N_CHUNKS, HOP).transpose(0, 2, 1)
    # weight: [514, 1, 400] -> wt_p[128 j + r, c] = w[c, 100 j + r]
    w2 = np.ascontiguousarray(w.reshape(C, WIN).T)  # [400, 514]
    wt = np.zeros((NJ * KP, C), dtype=ml_dtypes.bfloat16)
    for j in range(NJ):
        wt[KP * j : KP * j + HOP] = w2[HOP * j : HOP * (j + 1)]
    return xdev, wt


def kernel(x, weight):
    global _NC, LAST_RESULTS
    from concourse.bass_utils import run_bass_kernel_spmd

    _ensure_axon_hooks_stub()
    xdev, wt = _prep_inputs(x, weight)
    if _NC is None:
        _NC = build_program()
    in_maps = [
        {"x": np.ascontiguousarray(xdev[c * B_LOC : (c + 1) * B_LOC]), "wt": wt}
        for c in range(N_CORES)
    ]
    res = run_bass_kernel_spmd(_NC, in_maps, core_ids=list(range(N_CORES)))
    LAST_RESULTS = res
    # device emits [b_loc, 19, 128, 2, 514] bf16, frame = 256 gg + 128 h + f;
    # reorder, drop pad frames, transpose to [b, 514, 4803], upcast to f32
    outs = []
    for r in res.results:
        o = (
            r["out"]
            .astype(np.float32)
            .transpose(0, 1, 3, 2, 4)
            .reshape(B_LOC, N_FT * F_TILE, C)[:, :N_FRAMES, :]
        )
        outs.append(o.transpose(0, 2, 1))
    return np.ascontiguousarray(np.concatenate(outs, axis=0))
